# revision 28
# baseline (speedup 1.0000x reference)
"""Trainium2 Bass kernel for CMPGNN message passing (8-core SPMD), v3.

Sharding: nodes split contiguously across 8 cores (graph parallel).

Layer pipeline (software-pipelined per tile):
  for each target tile t of layer li:
      dma_gather (window A: sources on cores 0-3, window B: cores 4-7)
      per 128-edge block: hsel = OH1^T @ H4tile (TensorE), dot d =
      rowsum(G*hsel) (DVE), q = sigmoid(d) (ACT), m = q*W - G (DVE),
      agg += OH2^T @ m (TensorE, OH2 = -one-hot)
      hq_t = h1_t + agg; L2-normalize hq_t (per tile)
      dense for layer li+1 on tile t: transpose, h3/h4/h1 matmuls,
      stage [G|W] rows, DMA agin chunk
      at t==24: kick AllGather half-1 (rows 0..3200 of each slab)
      at t==T-1: kick AllGather half-2
Tables are double-buffered by layer parity so next-layer AllGathers can
overlap this layer's gathers. Window-A gathers of layer li+1 wait only
on half-1 (which completed during layer li), hiding collective latency.
Padding (dummy) edges gather row 0 and have zero OH1/OH2 columns/rows,
so they contribute nothing.
"""

import os
import sys
import math
import numpy as np

sys.path.insert(0, "/opt/trn_rl_repo")

from concourse import bass, bacc, mybir, tile  # noqa: E402
from concourse.masks import make_identity  # noqa: E402

AF = mybir.ActivationFunctionType
OP = mybir.AluOpType
DT = mybir.dt
AX = mybir.AxisListType

P = 128    # partitions == hidden size H
SPL = 3200  # per-core slab split row (25 tiles | 24 tiles)
USE_QRSQ = bool(int(os.environ.get("GNN_QRSQ", "1")))
FP8TAB = bool(int(os.environ.get("GNN_FP8TAB", "0")))


class Cfg:
    def __init__(self, N=50000, E=600000, F=500, H=128, C=40, KL=4, W=8):
        assert H == P
        self.N, self.E, self.F, self.H, self.C, self.KL, self.W = N, E, F, H, C, KL, W
        self.NL = N // W                       # owned nodes per core
        assert self.NL * W == N
        self.T = math.ceil((self.NL + 1) / P)  # node tiles per core
        self.NLp = self.T * P                  # padded nodes per core
        self.Fp = math.ceil(F / P) * P         # padded input features
        self.FC = self.Fp // P                 # input feature chunks
        self.SPL = SPL
        self.SPH = self.NLp - SPL              # second-half rows per slab
        self.NT1 = W * SPL                     # table1 rows
        self.NT2 = W * self.SPH                # table2 rows
        assert self.NT1 < 32768 and self.NT2 < 32768
        self.TSPL = SPL // P                   # tiles in first half (25)


def _wrap_idx16(flat):
    """dma_gather index layout: [i%16, i//16], replicated to 128 partitions."""
    n = flat.shape[0]
    assert n % 16 == 0
    blk = np.full((16, n // 16), -1, dtype=np.int16)
    blk[np.arange(n) % 16, np.arange(n) // 16] = flat.astype(np.int16)
    return np.tile(blk, (8, 1))


def plan(cfg, edge_index):
    """Host-side edge routing into per-(tile,window) 128-edge blocks."""
    W, NL, NLp, T = cfg.W, cfg.NL, cfg.NLp, cfg.T
    row = np.asarray(edge_index[0], dtype=np.int64)
    col = np.asarray(edge_index[1], dtype=np.int64)
    kk = row // NL                             # source core
    ju = row - kk * NL                         # source local index
    # window A: ju < SPL -> table1 row kk*SPL + ju
    # window B: ju >= SPL -> table2 row kk*SPH + (ju - SPL)
    winb = ju >= SPL
    srow = np.where(winb, kk * cfg.SPH + (ju - SPL), kk * SPL + ju)
    core_of = col // NL
    tile_of = (col - core_of * NL) // P
    part_of = (col - core_of * NL) % P

    nb_a = np.zeros((W, T), dtype=np.int64)
    nb_b = np.zeros((W, T), dtype=np.int64)
    edat = {}
    for k in range(W):
        mk = core_of == k
        for t in range(T):
            eids = np.nonzero(mk & (tile_of == t))[0]
            ea = eids[~winb[eids]]
            eb = eids[winb[eids]]
            nb_a[k, t] = (ea.shape[0] + P - 1) // P
            nb_b[k, t] = (eb.shape[0] + P - 1) // P
            edat[(k, t)] = (ea, eb)

    NB_a = [int(x) for x in nb_a.max(axis=0)]
    NB_b = [int(x) for x in nb_b.max(axis=0)]
    NBtot = int(sum(NB_a) + sum(NB_b))
    meta = dict(NB_a=NB_a, NB_b=NB_b, NBtot=NBtot)

    # gather-call chunk layout (<=8 blocks per call), shared across cores
    chunks = []          # (t, win, c0_blocks, nblocks)
    for t in range(T):
        for c0 in range(0, NB_a[t], 8):
            chunks.append((t, 0, c0, min(8, NB_a[t] - c0)))
        for c0 in range(0, NB_b[t], 8):
            chunks.append((t, 1, c0, min(8, NB_b[t] - c0)))
    meta["chunks"] = chunks

    per_core = []
    for k in range(W):
        ixa_l, ixb_l = [], []
        import ml_dtypes
        f8 = ml_dtypes.float8_e4m3
        oh1 = np.zeros((P, NBtot * P), dtype=f8)
        oh2 = np.zeros((P, NBtot * P), dtype=f8)
        b0 = 0
        for t in range(T):
            ea, eb = edat[(k, t)]
            sa = np.zeros(NB_a[t] * P, dtype=np.int64)
            sa[:ea.shape[0]] = srow[ea]
            sb = np.zeros(NB_b[t] * P, dtype=np.int64)
            sb[:eb.shape[0]] = srow[eb]
            ixa_l.append(sa)
            ixb_l.append(sb)
            for off, elist in ((0, ea), (NB_a[t] * P, eb)):
                for j, e in enumerate(elist):
                    sl = off + j
                    b = b0 + sl // P
                    oh1[part_of[e], b * P + sl % P] = 1.0
                    oh2[sl % P, b * P + part_of[e]] = -1.0
            b0 += NB_a[t] + NB_b[t]
        # per-core actual index counts per gather chunk
        gcnt = np.zeros(len(chunks), dtype=np.int32)
        for ci, (t, wn, c0, nbl) in enumerate(chunks):
            n_act = edat[(k, t)][wn].shape[0]
            # floor of 128 keeps all 16 SDMA engines' sem descriptors flowing;
            # rounded up to whole 128-blocks (ucode chunk granularity)
            c = min(max(n_act - c0 * P, P), nbl * P)
            gcnt[ci] = ((c + P - 1) // P) * P
        per_core.append(dict(
            ixa=_wrap_idx16(np.concatenate(ixa_l)),
            ixb=_wrap_idx16(np.concatenate(ixb_l)),
            oh1=oh1, oh2=oh2, gcnt=gcnt[None, :]))
    return meta, per_core


def build(cfg, meta, dtype16=DT.float16):
    """Build + compile the SPMD Tile kernel."""
    W, T, NLp, C, KL, FC = cfg.W, cfg.T, cfg.NLp, cfg.C, cfg.KL, cfg.FC
    f32 = DT.float32
    NB_a, NB_b = meta["NB_a"], meta["NB_b"]
    NB_t = [a + b for a, b in zip(NB_a, NB_b)]
    NBtot = meta["NBtot"]
    NBA, NBB = sum(NB_a), sum(NB_b)
    NBmax = max(NB_t)

    NQ = int(os.environ.get("GNN_NQ", "4"))
    nc = bacc.Bacc("TRN2", target_bir_lowering=False, debug=False,
                   num_devices=W, num_swdge_queues=NQ)

    xT = nc.dram_tensor("xT", [cfg.Fp, NLp], dtype16, kind="ExternalInput")
    WinT = nc.dram_tensor("WinT", [cfg.Fp, P], dtype16, kind="ExternalInput")
    W1T = nc.dram_tensor("W1T", [KL, P, P], dtype16, kind="ExternalInput")
    W2T = nc.dram_tensor("W2T", [KL, P, P], dtype16, kind="ExternalInput")
    WfT = nc.dram_tensor("WfT", [KL, P, P], dtype16, kind="ExternalInput")
    WoutT = nc.dram_tensor("WoutT", [P, C], dtype16, kind="ExternalInput")
    b_in_rep = nc.dram_tensor("b_in_rep", [P, P], f32, kind="ExternalInput")
    b_in_lst = nc.dram_tensor("b_in_lst", [P, P], f32, kind="ExternalInput")
    b_out_rep = nc.dram_tensor("b_out_rep", [P, C], f32, kind="ExternalInput")
    ixa = nc.dram_tensor("ixa", [P, NBA * 8], DT.int16, kind="ExternalInput")
    ixb = nc.dram_tensor("ixb", [P, NBB * 8], DT.int16, kind="ExternalInput")
    NCH = len(meta["chunks"])
    gcnt = nc.dram_tensor("gcnt", [1, NCH], DT.int32, kind="ExternalInput")
    oh1 = nc.dram_tensor("oh1", [P, NBtot * P], DT.float8e4,
                         kind="ExternalInput")
    oh2 = nc.dram_tensor("oh2", [P, NBtot * P], DT.float8e4,
                         kind="ExternalInput")
    out = nc.dram_tensor("out", [NLp, C], f32, kind="ExternalOutput")

    dt_tab = DT.float8e4 if FP8TAB else dtype16
    agin = nc.dram_tensor("agin", [NLp, 256], dt_tab)
    shared = "Shared" if W > 4 else "Local"
    tb1 = [nc.dram_tensor(f"tb1_{i}", [cfg.NT1, 256], dt_tab,
                          addr_space=shared) for i in range(2)]
    tb2 = [nc.dram_tensor(f"tb2_{i}", [cfg.NT2, 256], dt_tab,
                          addr_space=shared) for i in range(2)]

    agin_v = agin.ap().rearrange("(t p) f -> p t f", p=P)
    out_v = out.ap().rearrange("(t p) c -> p t c", p=P)

    with tile.TileContext(nc) as tc:
        with (
            tc.tile_pool(name="persist", bufs=1) as pp,
            tc.tile_pool(name="small", bufs=8) as mp,
            tc.tile_pool(name="psum", bufs=1, space="PSUM") as psp,
            tc.tile_pool(name="hselp", bufs=2, space="PSUM") as hp,
            tc.tile_pool(name="aggp", bufs=2, space="PSUM") as ap_,
        ):
            QT = pp.tile([P, T * P], dtype16, tag="QT")
            h1 = pp.tile([P, T * P], dtype16, tag="h1")
            h4sb = pp.tile([P, T * P], dtype16, tag="h4sb")
            uv = pp.tile([P, T * 256], dt_tab, tag="uv")   # [G|W] staging
            hq = pp.tile([P, T * P], f32, tag="hq")
            ident = pp.tile([P, P], f32, tag="ident")
            make_identity(nc, ident[:])

            sxa = pp.tile([P, NBA * 8], DT.int16, tag="sxa")
            sxb = pp.tile([P, NBB * 8], DT.int16, tag="sxb")
            scnt = pp.tile([1, NCH], DT.int32, tag="scnt")
            nc.sync.dma_start(out=sxa[:], in_=ixa[:, :])
            nc.sync.dma_start(out=sxb[:], in_=ixb[:, :])
            nc.sync.dma_start(out=scnt[:], in_=gcnt[:, :])

            binr = pp.tile([P, P], f32, tag="binr")
            binl = pp.tile([P, P], f32, tag="binl")
            boutr = pp.tile([P, C], f32, tag="boutr")
            w1 = pp.tile([P, KL * P], dtype16, tag="w1")
            w2 = pp.tile([P, KL * P], dtype16, tag="w2")
            wf = pp.tile([P, KL * P], dtype16, tag="wf")
            wo = pp.tile([P, C], dtype16, tag="wo")
            winT = pp.tile([P, FC * P], dtype16, tag="winT")
            ostage = pp.tile([P, T * C], f32, tag="ostage")

            nc.sync.dma_start(out=binr[:], in_=b_in_rep[:, :])
            nc.sync.dma_start(out=binl[:], in_=b_in_lst[:, :])
            nc.sync.dma_start(out=boutr[:], in_=b_out_rep[:, :])
            nc.sync.dma_start(out=wo[:], in_=WoutT[:, :])
            for l in range(KL):
                nc.sync.dma_start(out=w1[:, l * P:(l + 1) * P], in_=W1T[l, :, :])
                nc.sync.dma_start(out=w2[:, l * P:(l + 1) * P], in_=W2T[l, :, :])
                nc.sync.dma_start(out=wf[:, l * P:(l + 1) * P], in_=WfT[l, :, :])
            nc.sync.dma_start(
                out=winT[:].rearrange("p (c q) -> p c q", c=FC),
                in_=WinT.ap().rearrange("(c p) q -> p c q", p=P))

            def dense_tile(t, li):
                """h3/h4/h1 for layer li on tile t -> uv/h4sb/h1 + agin chunk."""
                tp = psp.tile([P, P], f32, space="PSUM", tag="tp")
                nc.tensor.transpose(out=tp[:], in_=hq[:, t * P:(t + 1) * P],
                                    identity=ident[:])
                nc.scalar.activation(out=QT[:, t * P:(t + 1) * P], in_=tp[:],
                                     func=AF.Copy)
                qt = QT[:, t * P:(t + 1) * P]
                p3 = psp.tile([P, P], f32, space="PSUM", tag="tp")
                p4 = psp.tile([P, P], f32, space="PSUM", tag="p4")
                nc.tensor.matmul(out=p3[:], lhsT=qt,
                                 rhs=w1[:, li * P:(li + 1) * P],
                                 start=True, stop=True)
                nc.tensor.matmul(out=p4[:], lhsT=qt,
                                 rhs=w2[:, li * P:(li + 1) * P],
                                 start=True, stop=True)
                gslice = uv[:, t * 256: t * 256 + P]
                nc.scalar.activation(out=gslice, in_=p3[:], func=AF.Copy)
                nc.scalar.activation(out=h4sb[:, t * P:(t + 1) * P],
                                     in_=p4[:], func=AF.Copy)
                pf = psp.tile([P, P], f32, space="PSUM", tag="p4")
                nc.tensor.matmul(out=pf[:], lhsT=qt,
                                 rhs=wf[:, li * P:(li + 1) * P],
                                 start=True, stop=True)
                nc.vector.tensor_tensor(
                    out=uv[:, t * 256 + P: t * 256 + 256],
                    in0=gslice, in1=h4sb[:, t * P:(t + 1) * P], op=OP.add)
                nc.scalar.activation(out=h1[:, t * P:(t + 1) * P],
                                     in_=pf[:], func=AF.Relu)
                nc.sync.dma_start(out=agin_v[:, t, :],
                                  in_=uv[:].rearrange(
                                      "p (s f) -> p s f", s=T)[:, t, :])

            def kick_collectives(part, li):
                pr = [list(range(W))]
                if part == 1:
                    nc.gpsimd.collective_compute(
                        "AllGather", OP.bypass, replica_groups=pr,
                        ins=[agin.ap()[0:cfg.SPL, :]],
                        outs=[tb1[li % 2].ap()])
                else:
                    nc.gpsimd.collective_compute(
                        "AllGather", OP.bypass, replica_groups=pr,
                        ins=[agin.ap()[cfg.SPL:NLp, :]],
                        outs=[tb2[li % 2].ap()])


            regs = {}
            for nbv in set(NB_a) | set(NB_b):
                for c0 in range(0, nbv, 8):
                    n_ = P * (min(c0 + 8, nbv) - c0)
                    if n_ > 0 and n_ not in regs:
                        regs[n_] = nc.gpsimd.to_reg(n_)

            # ---- phase 0: Q0 = x @ Win.T + b_in (into hq) ----
            with tc.tile_pool(name="ph0", bufs=1) as p0:
                xall = p0.tile([P, FC * NLp], dtype16, tag="xall")
                nc.sync.dma_start(
                    out=xall[:].rearrange("p (c n) -> p c n", c=FC),
                    in_=xT.ap().rearrange("(c p) n -> p c n", p=P))
                for t in range(T):
                    q0p = psp.tile([P, P], f32, space="PSUM", tag="tp")
                    for c in range(FC):
                        nc.tensor.matmul(
                            out=q0p[:],
                            lhsT=xall[:, c * NLp + t * P: c * NLp + (t + 1) * P],
                            rhs=winT[:, c * P:(c + 1) * P],
                            start=(c == 0), stop=(c == FC - 1))
                    nc.vector.tensor_tensor(
                        out=hq[:, t * P:(t + 1) * P], in0=q0p[:],
                        in1=(binr[:] if t < T - 1 else binl[:]), op=OP.add)
                    dense_tile(t, 0)
                    if t == cfg.TSPL - 1:
                        kick_collectives(1, 0)
            kick_collectives(2, 0)

            def seg_gather(dst3, dbase, src_ap, ixs, colbase, nb, qn):
                for c0 in range(0, nb, 8):
                    c1 = min(c0 + 8, nb)
                    n_ = P * (c1 - c0)
                    nc.gpsimd.dma_gather(
                        dst3[:, dbase + c0:dbase + c1, :], src_ap,
                        ixs[:, (colbase + c0) * 8: (colbase + c1) * 8],
                        n_, regs[n_], 256, queue_num=qn)

            # global block offsets for each (tile, window)
            blk0 = []
            acc = 0
            for t in range(T):
                blk0.append(acc)
                acc += NB_t[t]
            oa_off = [0] * T
            ob_off = [0] * T
            sa_ = sb_ = 0
            for t in range(T):
                oa_off[t] = sa_
                ob_off[t] = sb_
                sa_ += NB_a[t]
                sb_ += NB_b[t]

            NBAmax = max(NB_a)
            NBBmax = max(NB_b)
            LAG = 26

            def proc_win(t, li, win, gt, aggw, nbefore, nafter):
                """Gather + gating for one (tile, window); 4-block fusion.
                agg chain continues across windows: start on the first block
                (nbefore==0), stop on the last (nafter==0)."""
                nbw = NB_a[t] if win == 0 else NB_b[t]
                if nbw == 0:
                    return
                g3 = gt[:].rearrange("p (b f) -> p b f",
                                     b=(NBAmax if win == 0 else NBBmax))
                if win == 0:
                    seg_gather(g3, 0, tb1[li % 2].ap(), sxa, oa_off[t], nbw,
                               (t % 2) % NQ)
                    ob0 = blk0[t]
                else:
                    seg_gather(g3, 0, tb2[li % 2].ap(), sxb, ob_off[t], nbw,
                               (2 + t % 2) % NQ)
                    ob0 = blk0[t] + NB_a[t]
                o1w = op_.tile([P, max(NBAmax, NBBmax) * P], DT.float8e4,
                               tag="o1")
                o2w = op_.tile([P, max(NBAmax, NBBmax) * P], DT.float8e4,
                               tag="o2")
                nc.sync.dma_start(out=o1w[:, :nbw * P],
                                  in_=oh1[:, ob0 * P:(ob0 + nbw) * P])
                nc.sync.dma_start(out=o2w[:, :nbw * P],
                                  in_=oh2[:, ob0 * P:(ob0 + nbw) * P])
                g4v = gt[:].rearrange("p (b h f) -> p b h f", h=2, f=P)
                GB = 8
                for b0 in range(0, nbw, GB):
                    gsz = min(GB, nbw - b0)
                    hsel4 = hp.tile([P, GB * P], f32, space="PSUM", tag="hs")
                    for j in range(gsz):
                        nc.tensor.matmul(
                            out=hsel4[:, j * P:(j + 1) * P],
                            lhsT=o1w[:, (b0 + j) * P:(b0 + j + 1) * P],
                            rhs=h4sb[:, t * P:(t + 1) * P],
                            start=True, stop=True)
                    G4 = g4v[:, b0:b0 + gsz, 0, :]
                    W4 = g4v[:, b0:b0 + gsz, 1, :]
                    prod4 = mp.tile([P, GB * P], dtype16, tag="scr")
                    p4v = prod4[:, :gsz * P].rearrange("p (b f) -> p b f", b=gsz)
                    nc.vector.tensor_tensor(
                        out=p4v, in0=G4,
                        in1=hsel4[:, :gsz * P].rearrange(
                            "p (b f) -> p b f", b=gsz), op=OP.mult)
                    d4 = mp.tile([P, GB], f32, tag="d")
                    nc.vector.tensor_reduce(out=d4[:, :gsz], in_=p4v,
                                            axis=AX.X, op=OP.add)
                    q4 = mp.tile([P, GB], dtype16, tag="q")
                    nc.scalar.activation(out=q4[:, :gsz], in_=d4[:, :gsz],
                                         func=AF.Sigmoid)
                    m4 = mp.tile([P, GB * P], dtype16, tag="m")
                    m4v = m4[:, :gsz * P].rearrange("p (b f) -> p b f", b=gsz)
                    q4b = q4[:, :gsz].rearrange("p (b o) -> p b o", o=1)
                    nc.vector.tensor_tensor(
                        out=m4v, in0=W4, in1=q4b.to_broadcast([P, gsz, P]),
                        op=OP.mult)
                    nc.vector.tensor_tensor(out=m4v, in0=m4v, in1=G4,
                                            op=OP.subtract)
                    for j in range(gsz):
                        nc.tensor.matmul(
                            out=aggw[:],
                            lhsT=o2w[:, (b0 + j) * P:(b0 + j + 1) * P],
                            rhs=m4[:, j * P:(j + 1) * P],
                            start=(nbefore + b0 + j == 0),
                            stop=(nafter == 0 and b0 + j == nbw - 1))

            def tile_tail(t, li):
                hqt = hq[:, t * P:(t + 1) * P]
                nsq = mp.tile([P, P], f32, tag="nsq")
                sn = mp.tile([P, 1], f32, tag="sn")
                nc.vector.tensor_tensor(out=nsq[:], in0=hqt, in1=hqt,
                                        op=OP.mult)
                nc.vector.tensor_reduce(out=sn[:], in_=nsq[:],
                                        axis=AX.X, op=OP.add)
                rv = mp.tile([P, 1], f32, tag="rv")
                if USE_QRSQ:
                    # rv = rsqrt(sn): quake seed + 2 Newton iterations, all on
                    # DVE (scalar-engine Sqrt thrashes the act table against
                    # Sigmoid every tile)
                    nc.vector.tensor_scalar_max(out=sn[:], in0=sn[:],
                                                scalar1=1e-24)
                    ib = mp.tile([P, 1], DT.int32, tag="ib")
                    nc.vector.tensor_scalar(
                        out=ib[:], in0=sn[:].bitcast(DT.int32), scalar1=1,
                        scalar2=None, op0=OP.arith_shift_right)
                    nc.vector.tensor_scalar(
                        out=ib[:], in0=ib[:], scalar1=0x5F3759DF, scalar2=-1,
                        op0=OP.subtract, op1=OP.mult)
                    tN = mp.tile([P, 1], f32, tag="tN")
                    yv = ib[:].bitcast(f32)
                    for _ in range(2):
                        nc.vector.tensor_tensor(out=tN[:], in0=yv, in1=yv,
                                                op=OP.mult)
                        nc.vector.tensor_tensor(out=tN[:], in0=tN[:],
                                                in1=sn[:], op=OP.mult)
                        nc.vector.tensor_scalar(out=tN[:], in0=tN[:],
                                                scalar1=-0.5, scalar2=1.5,
                                                op0=OP.mult, op1=OP.add)
                        nc.vector.tensor_tensor(out=yv, in0=yv, in1=tN[:],
                                                op=OP.mult)
                    nc.vector.tensor_copy(out=rv[:], in_=yv)
                else:
                    nc.scalar.activation(out=sn[:], in_=sn[:], func=AF.Sqrt)
                    nc.vector.tensor_scalar_max(out=sn[:], in0=sn[:],
                                                scalar1=1e-12)
                    nc.vector.reciprocal(out=rv[:], in_=sn[:])
                nc.vector.tensor_tensor(out=hqt, in0=hqt,
                                        in1=rv[:].to_broadcast([P, P]),
                                        op=OP.mult)
                if li < KL - 1:
                    dense_tile(t, li + 1)
                    if t == cfg.TSPL - 1:
                        kick_collectives(1, li + 1)
                    elif t == T - 1:
                        kick_collectives(2, li + 1)
                else:
                    logits_tile(t)

            def logits_tile(t):
                tp = psp.tile([P, P], f32, space="PSUM", tag="tp")
                nc.tensor.transpose(out=tp[:], in_=hq[:, t * P:(t + 1) * P],
                                    identity=ident[:])
                nc.scalar.activation(out=QT[:, t * P:(t + 1) * P],
                                     in_=tp[:], func=AF.Copy)
                lp = psp.tile([P, C], f32, space="PSUM", tag="p4")
                nc.tensor.matmul(out=lp[:], lhsT=QT[:, t * P:(t + 1) * P],
                                 rhs=wo[:], start=True, stop=True)
                lg = mp.tile([P, C], f32, tag="lg")
                nc.vector.tensor_tensor(out=lg[:], in0=lp[:], in1=boutr[:],
                                        op=OP.add)
                mx = mp.tile([P, 1], f32, tag="mx")
                nc.vector.tensor_reduce(out=mx[:], in_=lg[:], axis=AX.X,
                                        op=OP.max)
                nmx = mp.tile([P, 1], f32, tag="nmx")
                nc.vector.tensor_scalar_mul(out=nmx[:], in0=mx[:],
                                            scalar1=-1.0)
                ex = mp.tile([P, C], f32, tag="ex")
                se = mp.tile([P, 1], f32, tag="se")
                nc.scalar.activation(out=ex[:], in_=lg[:], func=AF.Exp,
                                     bias=nmx[:], accum_out=se[:])
                nc.scalar.activation(out=se[:], in_=se[:], func=AF.Ln)
                nc.vector.tensor_tensor(out=mx[:], in0=mx[:], in1=se[:],
                                        op=OP.add)
                nc.vector.tensor_tensor(
                    out=ostage[:, t * C:(t + 1) * C], in0=lg[:],
                    in1=mx[:].to_broadcast([P, C]), op=OP.subtract)
                nc.sync.dma_start(out=out_v[:, t, :],
                                  in_=ostage[:, t * C:(t + 1) * C])

            with tc.tile_pool(name="gatA", bufs=4) as gpa, \
                 tc.tile_pool(name="gatB", bufs=4) as gpb, \
                 tc.tile_pool(name="ohp", bufs=4) as op_:
                for li in range(KL):
                    for t in range(T):
                        ga = gpa.tile([P, NBAmax * 256], dt_tab, tag="ga")
                        gb = gpb.tile([P, NBBmax * 256], dt_tab, tag="gb")
                        aggw = ap_.tile([P, P], f32, space="PSUM", tag="agg")
                        proc_win(t, li, 0, ga, aggw, 0, NB_b[t])
                        proc_win(t, li, 1, gb, aggw, NB_a[t], 0)
                        nc.vector.tensor_tensor(
                            out=hq[:, t * P:(t + 1) * P],
                            in0=h1[:, t * P:(t + 1) * P], in1=aggw[:],
                            op=OP.add)
                        tile_tail(t, li)

    nc.compile()
    return nc


def host_inputs(cfg, meta, per_core, inputs, np16=np.float16):
    x = np.asarray(inputs["x"], np.float32)
    W, NL = cfg.W, cfg.NL
    WinT = np.zeros((cfg.Fp, P), np16)
    WinT[:cfg.F] = np.asarray(inputs["W_in"], np.float32).T.astype(np16)
    W1T = np.ascontiguousarray(
        np.asarray(inputs["W1"], np.float32).transpose(0, 2, 1)).astype(np16)
    W2T = np.ascontiguousarray(
        np.asarray(inputs["W2"], np.float32).transpose(0, 2, 1)).astype(np16)
    WfT = np.ascontiguousarray(
        np.asarray(inputs["Wf"], np.float32).transpose(0, 2, 1)).astype(np16)
    WoT = np.ascontiguousarray(
        np.asarray(inputs["W_out"], np.float32).T).astype(np16)
    binr = np.tile(np.asarray(inputs["b_in"], np.float32)[None, :], (P, 1))
    boutr = np.tile(np.asarray(inputs["b_out"], np.float32)[None, :], (P, 1))
    maps = []
    for k in range(W):
        xk = np.zeros((cfg.Fp, cfg.NLp), np16)
        xk[:cfg.F, :NL] = x[k * NL:(k + 1) * NL].T.astype(np16)
        binl = binr.copy()
        binl[NL % P if NL % P else 0:] = 0.0   # pad-node rows of last tile
        m = dict(
            xT=xk, WinT=WinT, W1T=W1T, W2T=W2T, WfT=WfT, WoutT=WoT,
            b_in_rep=np.ascontiguousarray(binr, np.float32),
            b_in_lst=np.ascontiguousarray(binl, np.float32),
            b_out_rep=np.ascontiguousarray(boutr, np.float32),
            ixa=per_core[k]["ixa"], ixb=per_core[k]["ixb"],
            oh1=per_core[k]["oh1"], oh2=per_core[k]["oh2"],
            gcnt=per_core[k]["gcnt"])
        maps.append(m)
    return maps


def _install_profile_hook():
    """Provide antenv.axon_hooks (absent in this image) so that
    run_bass_kernel_spmd(trace=True) can collect an NTFF profile."""
    try:
        import types
        import antenv
        if "antenv.axon_hooks" not in sys.modules:
            mod = types.ModuleType("antenv.axon_hooks")
            state = {"hook": None}
            mod.set_axon_ntff_profile_hook = lambda h: state.__setitem__("hook", h)
            mod.get_axon_ntff_profile_hook = lambda: state["hook"]
            sys.modules["antenv.axon_hooks"] = mod
            antenv.axon_hooks = mod
        from antenv.axon_hooks import (get_axon_ntff_profile_hook,
                                       set_axon_ntff_profile_hook)
        if get_axon_ntff_profile_hook() is None:
            from trn_agent_boot.trn_boot import _ntff_profile_via_ctypes
            set_axon_ntff_profile_hook(
                _ntff_profile_via_ctypes("/opt/axon/libaxon_pjrt.so"))
        return True
    except Exception as e:  # degrade to untraced run
        print(f"profile hook unavailable: {e}")
        return False


def kernel(**inputs):
    cfg = Cfg()
    edge_index = np.asarray(inputs["edge_index"])
    meta, per_core = plan(cfg, edge_index)
    nc = build(cfg, meta)
    in_maps = host_inputs(cfg, meta, per_core, inputs)
    trace = bool(int(os.environ.get("GNN_TRACE", "0")))
    if trace:
        trace = _install_profile_hook()
    from concourse import bass_utils
    res = bass_utils.run_bass_kernel_spmd(
        nc, in_maps, core_ids=list(range(cfg.W)), trace=trace)
    if res.exec_time_ns is not None:
        print(f"HW exec time: {res.exec_time_ns} ns")
    outs = [res.results[k]["out"][:cfg.NL] for k in range(cfg.W)]
    return np.concatenate(outs, axis=0).astype(np.float32)



# revision 32
# speedup vs baseline: 1.1753x; 1.1753x over previous
"""Trainium2 Bass kernel for CMPGNN message passing (8-core SPMD), v3.

Sharding: nodes split contiguously across 8 cores (graph parallel).

Layer pipeline (software-pipelined per tile):
  for each target tile t of layer li:
      dma_gather (window A: sources on cores 0-3, window B: cores 4-7)
      per 128-edge block: hsel = OH1^T @ H4tile (TensorE), dot d =
      rowsum(G*hsel) (DVE), q = sigmoid(d) (ACT), m = q*W - G (DVE),
      agg += OH2^T @ m (TensorE, OH2 = -one-hot)
      hq_t = h1_t + agg; L2-normalize hq_t (per tile)
      dense for layer li+1 on tile t: transpose, h3/h4/h1 matmuls,
      stage [G|W] rows, DMA agin chunk
      at t==24: kick AllGather half-1 (rows 0..3200 of each slab)
      at t==T-1: kick AllGather half-2
Tables are double-buffered by layer parity so next-layer AllGathers can
overlap this layer's gathers. Window-A gathers of layer li+1 wait only
on half-1 (which completed during layer li), hiding collective latency.
Padding (dummy) edges gather row 0 and have zero OH1/OH2 columns/rows,
so they contribute nothing.
"""

import os
import sys
import math
import numpy as np

sys.path.insert(0, "/opt/trn_rl_repo")

from concourse import bass, bacc, mybir, tile  # noqa: E402
from concourse.masks import make_identity  # noqa: E402

AF = mybir.ActivationFunctionType
OP = mybir.AluOpType
DT = mybir.dt
AX = mybir.AxisListType

P = 128    # partitions == hidden size H
SPL = 4000  # per-core slab split row: big window A (rows < SPL) allgathers
            # early; small window B tail minimizes the layer-boundary bubble
USE_QRSQ = bool(int(os.environ.get("GNN_QRSQ", "1")))
FP8TAB = bool(int(os.environ.get("GNN_FP8TAB", "0")))


class Cfg:
    def __init__(self, N=50000, E=600000, F=500, H=128, C=40, KL=4, W=8):
        assert H == P
        self.N, self.E, self.F, self.H, self.C, self.KL, self.W = N, E, F, H, C, KL, W
        self.NL = N // W                       # owned nodes per core
        assert self.NL * W == N
        self.T = math.ceil((self.NL + 1) / P)  # node tiles per core
        self.NLp = self.T * P                  # padded nodes per core
        self.Fp = math.ceil(F / P) * P         # padded input features
        self.FC = self.Fp // P                 # input feature chunks
        self.SPL = SPL
        self.SPH = self.NLp - SPL              # second-half rows per slab
        self.NT1 = W * SPL                     # table1 rows
        self.NT2 = W * self.SPH                # table2 rows
        assert self.NT1 < 32768 and self.NT2 < 32768
        self.TSPL = math.ceil(SPL / P)         # tiles covering window A rows


def _wrap_idx16(flat):
    """dma_gather index layout: [i%16, i//16], replicated to 128 partitions."""
    n = flat.shape[0]
    assert n % 16 == 0
    blk = np.full((16, n // 16), -1, dtype=np.int16)
    blk[np.arange(n) % 16, np.arange(n) // 16] = flat.astype(np.int16)
    return np.tile(blk, (8, 1))


def plan(cfg, edge_index):
    """Host-side edge routing into per-(tile,window) 128-edge blocks."""
    W, NL, NLp, T = cfg.W, cfg.NL, cfg.NLp, cfg.T
    row = np.asarray(edge_index[0], dtype=np.int64)
    col = np.asarray(edge_index[1], dtype=np.int64)
    kk = row // NL                             # source core
    ju = row - kk * NL                         # source local index
    # window A: ju < SPL -> table1 row kk*SPL + ju
    # window B: ju >= SPL -> table2 row kk*SPH + (ju - SPL)
    winb = ju >= SPL
    srow = np.where(winb, kk * cfg.SPH + (ju - SPL), kk * SPL + ju)
    core_of = col // NL
    tile_of = (col - core_of * NL) // P
    part_of = (col - core_of * NL) % P

    nb_a = np.zeros((W, T), dtype=np.int64)
    nb_b = np.zeros((W, T), dtype=np.int64)
    edat = {}
    for k in range(W):
        mk = core_of == k
        for t in range(T):
            eids = np.nonzero(mk & (tile_of == t))[0]
            ea = eids[~winb[eids]]
            eb = eids[winb[eids]]
            nb_a[k, t] = (ea.shape[0] + P - 1) // P
            nb_b[k, t] = (eb.shape[0] + P - 1) // P
            edat[(k, t)] = (ea, eb)

    NB_a = [int(x) for x in nb_a.max(axis=0)]
    NB_b = [int(x) for x in nb_b.max(axis=0)]
    NBtot = int(sum(NB_a) + sum(NB_b))
    meta = dict(NB_a=NB_a, NB_b=NB_b, NBtot=NBtot)

    # gather-call chunk layout (<=8 blocks per call), shared across cores
    chunks = []          # (t, win, c0_blocks, nblocks)
    for t in range(T):
        for c0 in range(0, NB_a[t], 8):
            chunks.append((t, 0, c0, min(8, NB_a[t] - c0)))
        for c0 in range(0, NB_b[t], 8):
            chunks.append((t, 1, c0, min(8, NB_b[t] - c0)))
    meta["chunks"] = chunks

    per_core = []
    for k in range(W):
        ixa_l, ixb_l = [], []
        import ml_dtypes
        f8 = ml_dtypes.float8_e4m3
        oh1 = np.zeros((P, NBtot * P), dtype=f8)
        oh2 = np.zeros((P, NBtot * P), dtype=f8)
        b0 = 0
        for t in range(T):
            ea, eb = edat[(k, t)]
            sa = np.zeros(NB_a[t] * P, dtype=np.int64)
            sa[:ea.shape[0]] = srow[ea]
            sb = np.zeros(NB_b[t] * P, dtype=np.int64)
            sb[:eb.shape[0]] = srow[eb]
            ixa_l.append(sa)
            ixb_l.append(sb)
            for off, elist in ((0, ea), (NB_a[t] * P, eb)):
                for j, e in enumerate(elist):
                    sl = off + j
                    b = b0 + sl // P
                    oh1[part_of[e], b * P + sl % P] = 1.0
                    oh2[sl % P, b * P + part_of[e]] = -1.0
            b0 += NB_a[t] + NB_b[t]
        # per-core actual index counts per gather chunk
        gcnt = np.zeros(len(chunks), dtype=np.int32)
        for ci, (t, wn, c0, nbl) in enumerate(chunks):
            n_act = edat[(k, t)][wn].shape[0]
            # floor of 128 keeps all 16 SDMA engines' sem descriptors flowing;
            # rounded up to whole 128-blocks (ucode chunk granularity)
            c = min(max(n_act - c0 * P, P), nbl * P)
            gcnt[ci] = ((c + P - 1) // P) * P
        per_core.append(dict(
            ixa=_wrap_idx16(np.concatenate(ixa_l)),
            ixb=_wrap_idx16(np.concatenate(ixb_l)),
            oh1=oh1, oh2=oh2, gcnt=gcnt[None, :]))
    return meta, per_core


def build(cfg, meta, dtype16=DT.float16):
    """Build + compile the SPMD Tile kernel."""
    W, T, NLp, C, KL, FC = cfg.W, cfg.T, cfg.NLp, cfg.C, cfg.KL, cfg.FC
    f32 = DT.float32
    NB_a, NB_b = meta["NB_a"], meta["NB_b"]
    NB_t = [a + b for a, b in zip(NB_a, NB_b)]
    NBtot = meta["NBtot"]
    NBA, NBB = sum(NB_a), sum(NB_b)
    NBmax = max(NB_t)

    NQ = int(os.environ.get("GNN_NQ", "4"))
    nc = bacc.Bacc("TRN2", target_bir_lowering=False, debug=False,
                   num_devices=W, num_swdge_queues=NQ)

    xT = nc.dram_tensor("xT", [cfg.Fp, NLp], dtype16, kind="ExternalInput")
    WinT = nc.dram_tensor("WinT", [cfg.Fp, P], dtype16, kind="ExternalInput")
    W1T = nc.dram_tensor("W1T", [KL, P, P], dtype16, kind="ExternalInput")
    W2T = nc.dram_tensor("W2T", [KL, P, P], dtype16, kind="ExternalInput")
    WfT = nc.dram_tensor("WfT", [KL, P, P], dtype16, kind="ExternalInput")
    WoutT = nc.dram_tensor("WoutT", [P, C], dtype16, kind="ExternalInput")
    b_in_rep = nc.dram_tensor("b_in_rep", [P, P], f32, kind="ExternalInput")
    b_in_lst = nc.dram_tensor("b_in_lst", [P, P], f32, kind="ExternalInput")
    b_out_rep = nc.dram_tensor("b_out_rep", [P, C], f32, kind="ExternalInput")
    ixa = nc.dram_tensor("ixa", [P, NBA * 8], DT.int16, kind="ExternalInput")
    ixb = nc.dram_tensor("ixb", [P, NBB * 8], DT.int16, kind="ExternalInput")
    NCH = len(meta["chunks"])
    gcnt = nc.dram_tensor("gcnt", [1, NCH], DT.int32, kind="ExternalInput")
    oh1 = nc.dram_tensor("oh1", [P, NBtot * P], DT.float8e4,
                         kind="ExternalInput")
    oh2 = nc.dram_tensor("oh2", [P, NBtot * P], DT.float8e4,
                         kind="ExternalInput")
    out = nc.dram_tensor("out", [NLp, C], f32, kind="ExternalOutput")

    dt_tab = DT.float8e4 if FP8TAB else dtype16
    agin = nc.dram_tensor("agin", [NLp, 256], dt_tab)
    shared = "Shared" if W > 4 else "Local"
    tb1 = [nc.dram_tensor(f"tb1_{i}", [cfg.NT1, 256], dt_tab,
                          addr_space=shared) for i in range(2)]
    tb2 = [nc.dram_tensor(f"tb2_{i}", [cfg.NT2, 256], dt_tab,
                          addr_space=shared) for i in range(2)]

    agin_v = agin.ap().rearrange("(t p) f -> p t f", p=P)
    out_v = out.ap().rearrange("(t p) c -> p t c", p=P)

    with tile.TileContext(nc) as tc:
        with (
            tc.tile_pool(name="persist", bufs=1) as pp,
            tc.tile_pool(name="small", bufs=8) as mp,
            tc.tile_pool(name="psum", bufs=1, space="PSUM") as psp,
            tc.tile_pool(name="hselp", bufs=2, space="PSUM") as hp,
            tc.tile_pool(name="aggp", bufs=2, space="PSUM") as ap_,
        ):
            QT = pp.tile([P, T * P], dtype16, tag="QT")
            h1 = pp.tile([P, T * P], dtype16, tag="h1")
            h4sb = pp.tile([P, T * P], dtype16, tag="h4sb")
            uv = pp.tile([P, T * 256], dt_tab, tag="uv")   # [G|W] staging
            hq = pp.tile([P, T * P], f32, tag="hq")
            ident = pp.tile([P, P], f32, tag="ident")
            make_identity(nc, ident[:])

            sxa = pp.tile([P, NBA * 8], DT.int16, tag="sxa")
            sxb = pp.tile([P, NBB * 8], DT.int16, tag="sxb")
            scnt = pp.tile([1, NCH], DT.int32, tag="scnt")
            nc.sync.dma_start(out=sxa[:], in_=ixa[:, :])
            nc.sync.dma_start(out=sxb[:], in_=ixb[:, :])
            nc.sync.dma_start(out=scnt[:], in_=gcnt[:, :])

            binr = pp.tile([P, P], f32, tag="binr")
            binl = pp.tile([P, P], f32, tag="binl")
            boutr = pp.tile([P, C], f32, tag="boutr")
            w1 = pp.tile([P, KL * P], dtype16, tag="w1")
            w2 = pp.tile([P, KL * P], dtype16, tag="w2")
            wf = pp.tile([P, KL * P], dtype16, tag="wf")
            wo = pp.tile([P, C], dtype16, tag="wo")
            winT = pp.tile([P, FC * P], dtype16, tag="winT")
            ostage = pp.tile([P, T * C], f32, tag="ostage")

            nc.sync.dma_start(out=binr[:], in_=b_in_rep[:, :])
            nc.sync.dma_start(out=binl[:], in_=b_in_lst[:, :])
            nc.sync.dma_start(out=boutr[:], in_=b_out_rep[:, :])
            nc.sync.dma_start(out=wo[:], in_=WoutT[:, :])
            for l in range(KL):
                nc.sync.dma_start(out=w1[:, l * P:(l + 1) * P], in_=W1T[l, :, :])
                nc.sync.dma_start(out=w2[:, l * P:(l + 1) * P], in_=W2T[l, :, :])
                nc.sync.dma_start(out=wf[:, l * P:(l + 1) * P], in_=WfT[l, :, :])
            nc.sync.dma_start(
                out=winT[:].rearrange("p (c q) -> p c q", c=FC),
                in_=WinT.ap().rearrange("(c p) q -> p c q", p=P))

            def dense_tile(t, li):
                """h3/h4/h1 for layer li on tile t -> uv/h4sb/h1 + agin chunk."""
                tp = psp.tile([P, P], f32, space="PSUM", tag="tp")
                nc.tensor.transpose(out=tp[:], in_=hq[:, t * P:(t + 1) * P],
                                    identity=ident[:])
                nc.scalar.activation(out=QT[:, t * P:(t + 1) * P], in_=tp[:],
                                     func=AF.Copy)
                qt = QT[:, t * P:(t + 1) * P]
                p3 = psp.tile([P, P], f32, space="PSUM", tag="tp")
                p4 = psp.tile([P, P], f32, space="PSUM", tag="p4")
                nc.tensor.matmul(out=p3[:], lhsT=qt,
                                 rhs=w1[:, li * P:(li + 1) * P],
                                 start=True, stop=True)
                nc.tensor.matmul(out=p4[:], lhsT=qt,
                                 rhs=w2[:, li * P:(li + 1) * P],
                                 start=True, stop=True)
                gslice = uv[:, t * 256: t * 256 + P]
                nc.scalar.activation(out=gslice, in_=p3[:], func=AF.Copy)
                nc.scalar.activation(out=h4sb[:, t * P:(t + 1) * P],
                                     in_=p4[:], func=AF.Copy)
                pf = psp.tile([P, P], f32, space="PSUM", tag="p4")
                nc.tensor.matmul(out=pf[:], lhsT=qt,
                                 rhs=wf[:, li * P:(li + 1) * P],
                                 start=True, stop=True)
                nc.vector.tensor_tensor(
                    out=uv[:, t * 256 + P: t * 256 + 256],
                    in0=gslice, in1=h4sb[:, t * P:(t + 1) * P], op=OP.add)
                nc.scalar.activation(out=h1[:, t * P:(t + 1) * P],
                                     in_=pf[:], func=AF.Relu)
                nc.sync.dma_start(out=agin_v[:, t, :],
                                  in_=uv[:].rearrange(
                                      "p (s f) -> p s f", s=T)[:, t, :])

            def kick_collectives(part, li):
                pr = [list(range(W))]
                if part == 1:
                    nc.gpsimd.collective_compute(
                        "AllGather", OP.bypass, replica_groups=pr,
                        ins=[agin.ap()[0:cfg.SPL, :]],
                        outs=[tb1[li % 2].ap()])
                else:
                    nc.gpsimd.collective_compute(
                        "AllGather", OP.bypass, replica_groups=pr,
                        ins=[agin.ap()[cfg.SPL:NLp, :]],
                        outs=[tb2[li % 2].ap()])


            qrr = [0]
            regs = {}
            for nbv in set(NB_a) | set(NB_b):
                for c0 in range(0, nbv, 8):
                    n_ = P * (min(c0 + 8, nbv) - c0)
                    if n_ > 0 and n_ not in regs:
                        regs[n_] = nc.gpsimd.to_reg(n_)

            # ---- phase 0: Q0 = x @ Win.T + b_in (into hq) ----
            with tc.tile_pool(name="ph0", bufs=1) as p0:
                xall = p0.tile([P, FC * NLp], dtype16, tag="xall")
                nc.sync.dma_start(
                    out=xall[:].rearrange("p (c n) -> p c n", c=FC),
                    in_=xT.ap().rearrange("(c p) n -> p c n", p=P))
                for t in range(T):
                    q0p = psp.tile([P, P], f32, space="PSUM", tag="tp")
                    for c in range(FC):
                        nc.tensor.matmul(
                            out=q0p[:],
                            lhsT=xall[:, c * NLp + t * P: c * NLp + (t + 1) * P],
                            rhs=winT[:, c * P:(c + 1) * P],
                            start=(c == 0), stop=(c == FC - 1))
                    nc.vector.tensor_tensor(
                        out=hq[:, t * P:(t + 1) * P], in0=q0p[:],
                        in1=(binr[:] if t < T - 1 else binl[:]), op=OP.add)
                    dense_tile(t, 0)
                    if t == cfg.TSPL - 1:
                        kick_collectives(1, 0)
            kick_collectives(2, 0)

            def seg_gather(dst3, dbase, src_ap, ixs, colbase, nb, qn):
                for c0 in range(0, nb, 8):
                    c1 = min(c0 + 8, nb)
                    n_ = P * (c1 - c0)
                    nc.gpsimd.dma_gather(
                        dst3[:, dbase + c0:dbase + c1, :], src_ap,
                        ixs[:, (colbase + c0) * 8: (colbase + c1) * 8],
                        n_, regs[n_], 256, queue_num=qn)

            # global block offsets for each (tile, window)
            blk0 = []
            acc = 0
            for t in range(T):
                blk0.append(acc)
                acc += NB_t[t]
            oa_off = [0] * T
            ob_off = [0] * T
            sa_ = sb_ = 0
            for t in range(T):
                oa_off[t] = sa_
                ob_off[t] = sb_
                sa_ += NB_a[t]
                sb_ += NB_b[t]

            NBAmax = max(NB_a)
            NBBmax = max(NB_b)
            LAG = 26

            def proc_win(t, li, win, gt, aggw, nbefore, nafter):
                """Gather + gating for one (tile, window); 4-block fusion.
                agg chain continues across windows: start on the first block
                (nbefore==0), stop on the last (nafter==0)."""
                nbw = NB_a[t] if win == 0 else NB_b[t]
                if nbw == 0:
                    return
                g3 = gt[:].rearrange("p (b f) -> p b f",
                                     b=(NBAmax if win == 0 else NBBmax))
                qn = qrr[0] % NQ
                qrr[0] += 1
                if win == 0:
                    seg_gather(g3, 0, tb1[li % 2].ap(), sxa, oa_off[t], nbw,
                               qn)
                    ob0 = blk0[t]
                else:
                    seg_gather(g3, 0, tb2[li % 2].ap(), sxb, ob_off[t], nbw,
                               qn)
                    ob0 = blk0[t] + NB_a[t]
                o1w = op_.tile([P, max(NBAmax, NBBmax) * P], DT.float8e4,
                               tag="o1")
                o2w = op_.tile([P, max(NBAmax, NBBmax) * P], DT.float8e4,
                               tag="o2")
                nc.sync.dma_start(out=o1w[:, :nbw * P],
                                  in_=oh1[:, ob0 * P:(ob0 + nbw) * P])
                nc.sync.dma_start(out=o2w[:, :nbw * P],
                                  in_=oh2[:, ob0 * P:(ob0 + nbw) * P])
                g4v = gt[:].rearrange("p (b h f) -> p b h f", h=2, f=P)
                GB = 8
                for b0 in range(0, nbw, GB):
                    gsz = min(GB, nbw - b0)
                    hsel4 = hp.tile([P, GB * P], f32, space="PSUM", tag="hs")
                    for j in range(gsz):
                        nc.tensor.matmul(
                            out=hsel4[:, j * P:(j + 1) * P],
                            lhsT=o1w[:, (b0 + j) * P:(b0 + j + 1) * P],
                            rhs=h4sb[:, t * P:(t + 1) * P],
                            start=True, stop=True)
                    G4 = g4v[:, b0:b0 + gsz, 0, :]
                    W4 = g4v[:, b0:b0 + gsz, 1, :]
                    prod4 = mp.tile([P, GB * P], dtype16, tag="scr")
                    p4v = prod4[:, :gsz * P].rearrange("p (b f) -> p b f", b=gsz)
                    nc.vector.tensor_tensor(
                        out=p4v, in0=G4,
                        in1=hsel4[:, :gsz * P].rearrange(
                            "p (b f) -> p b f", b=gsz), op=OP.mult)
                    d4 = mp.tile([P, GB], f32, tag="d")
                    nc.vector.tensor_reduce(out=d4[:, :gsz], in_=p4v,
                                            axis=AX.X, op=OP.add)
                    q4 = mp.tile([P, GB], dtype16, tag="q")
                    nc.scalar.activation(out=q4[:, :gsz], in_=d4[:, :gsz],
                                         func=AF.Sigmoid)
                    m4 = mp.tile([P, GB * P], dtype16, tag="m")
                    m4v = m4[:, :gsz * P].rearrange("p (b f) -> p b f", b=gsz)
                    q4b = q4[:, :gsz].rearrange("p (b o) -> p b o", o=1)
                    nc.vector.tensor_tensor(
                        out=m4v, in0=W4, in1=q4b.to_broadcast([P, gsz, P]),
                        op=OP.mult)
                    nc.vector.tensor_tensor(out=m4v, in0=m4v, in1=G4,
                                            op=OP.subtract)
                    for j in range(gsz):
                        nc.tensor.matmul(
                            out=aggw[:],
                            lhsT=o2w[:, (b0 + j) * P:(b0 + j + 1) * P],
                            rhs=m4[:, j * P:(j + 1) * P],
                            start=(nbefore + b0 + j == 0),
                            stop=(nafter == 0 and b0 + j == nbw - 1))

            def tile_tail(t, li):
                hqt = hq[:, t * P:(t + 1) * P]
                nsq = mp.tile([P, P], f32, tag="nsq")
                sn = mp.tile([P, 1], f32, tag="sn")
                nc.vector.tensor_tensor(out=nsq[:], in0=hqt, in1=hqt,
                                        op=OP.mult)
                nc.vector.tensor_reduce(out=sn[:], in_=nsq[:],
                                        axis=AX.X, op=OP.add)
                rv = mp.tile([P, 1], f32, tag="rv")
                if USE_QRSQ:
                    # rv = rsqrt(sn): quake seed + 2 Newton iterations, all on
                    # DVE (scalar-engine Sqrt thrashes the act table against
                    # Sigmoid every tile)
                    nc.vector.tensor_scalar_max(out=sn[:], in0=sn[:],
                                                scalar1=1e-24)
                    ib = mp.tile([P, 1], DT.int32, tag="ib")
                    nc.vector.tensor_scalar(
                        out=ib[:], in0=sn[:].bitcast(DT.int32), scalar1=1,
                        scalar2=None, op0=OP.arith_shift_right)
                    nc.vector.tensor_scalar(
                        out=ib[:], in0=ib[:], scalar1=0x5F3759DF, scalar2=-1,
                        op0=OP.subtract, op1=OP.mult)
                    tN = mp.tile([P, 1], f32, tag="tN")
                    yv = ib[:].bitcast(f32)
                    for _ in range(2):
                        nc.vector.tensor_tensor(out=tN[:], in0=yv, in1=yv,
                                                op=OP.mult)
                        nc.vector.tensor_tensor(out=tN[:], in0=tN[:],
                                                in1=sn[:], op=OP.mult)
                        nc.vector.tensor_scalar(out=tN[:], in0=tN[:],
                                                scalar1=-0.5, scalar2=1.5,
                                                op0=OP.mult, op1=OP.add)
                        nc.vector.tensor_tensor(out=yv, in0=yv, in1=tN[:],
                                                op=OP.mult)
                    nc.vector.tensor_copy(out=rv[:], in_=yv)
                else:
                    nc.scalar.activation(out=sn[:], in_=sn[:], func=AF.Sqrt)
                    nc.vector.tensor_scalar_max(out=sn[:], in0=sn[:],
                                                scalar1=1e-12)
                    nc.vector.reciprocal(out=rv[:], in_=sn[:])
                nc.vector.tensor_tensor(out=hqt, in0=hqt,
                                        in1=rv[:].to_broadcast([P, P]),
                                        op=OP.mult)
                if li < KL - 1:
                    dense_tile(t, li + 1)
                    if t == cfg.TSPL - 1:
                        kick_collectives(1, li + 1)
                    elif t == T - 1:
                        kick_collectives(2, li + 1)
                else:
                    logits_tile(t)

            def logits_tile(t):
                tp = psp.tile([P, P], f32, space="PSUM", tag="tp")
                nc.tensor.transpose(out=tp[:], in_=hq[:, t * P:(t + 1) * P],
                                    identity=ident[:])
                nc.scalar.activation(out=QT[:, t * P:(t + 1) * P],
                                     in_=tp[:], func=AF.Copy)
                lp = psp.tile([P, C], f32, space="PSUM", tag="p4")
                nc.tensor.matmul(out=lp[:], lhsT=QT[:, t * P:(t + 1) * P],
                                 rhs=wo[:], start=True, stop=True)
                lg = mp.tile([P, C], f32, tag="lg")
                nc.vector.tensor_tensor(out=lg[:], in0=lp[:], in1=boutr[:],
                                        op=OP.add)
                mx = mp.tile([P, 1], f32, tag="mx")
                nc.vector.tensor_reduce(out=mx[:], in_=lg[:], axis=AX.X,
                                        op=OP.max)
                nmx = mp.tile([P, 1], f32, tag="nmx")
                nc.vector.tensor_scalar_mul(out=nmx[:], in0=mx[:],
                                            scalar1=-1.0)
                ex = mp.tile([P, C], f32, tag="ex")
                se = mp.tile([P, 1], f32, tag="se")
                nc.scalar.activation(out=ex[:], in_=lg[:], func=AF.Exp,
                                     bias=nmx[:], accum_out=se[:])
                nc.scalar.activation(out=se[:], in_=se[:], func=AF.Ln)
                nc.vector.tensor_tensor(out=mx[:], in0=mx[:], in1=se[:],
                                        op=OP.add)
                nc.vector.tensor_tensor(
                    out=ostage[:, t * C:(t + 1) * C], in0=lg[:],
                    in1=mx[:].to_broadcast([P, C]), op=OP.subtract)
                nc.sync.dma_start(out=out_v[:, t, :],
                                  in_=ostage[:, t * C:(t + 1) * C])

            with tc.tile_pool(name="gatA", bufs=4) as gpa, \
                 tc.tile_pool(name="gatB", bufs=4) as gpb, \
                 tc.tile_pool(name="ohp", bufs=4) as op_:
                for li in range(KL):
                    for t in range(T):
                        ga = gpa.tile([P, NBAmax * 256], dt_tab, tag="ga")
                        gb = gpb.tile([P, NBBmax * 256], dt_tab, tag="gb")
                        aggw = ap_.tile([P, P], f32, space="PSUM", tag="agg")
                        proc_win(t, li, 0, ga, aggw, 0, NB_b[t])
                        proc_win(t, li, 1, gb, aggw, NB_a[t], 0)
                        nc.vector.tensor_tensor(
                            out=hq[:, t * P:(t + 1) * P],
                            in0=h1[:, t * P:(t + 1) * P], in1=aggw[:],
                            op=OP.add)
                        tile_tail(t, li)

    nc.compile()
    return nc


def host_inputs(cfg, meta, per_core, inputs, np16=np.float16):
    x = np.asarray(inputs["x"], np.float32)
    W, NL = cfg.W, cfg.NL
    WinT = np.zeros((cfg.Fp, P), np16)
    WinT[:cfg.F] = np.asarray(inputs["W_in"], np.float32).T.astype(np16)
    W1T = np.ascontiguousarray(
        np.asarray(inputs["W1"], np.float32).transpose(0, 2, 1)).astype(np16)
    W2T = np.ascontiguousarray(
        np.asarray(inputs["W2"], np.float32).transpose(0, 2, 1)).astype(np16)
    WfT = np.ascontiguousarray(
        np.asarray(inputs["Wf"], np.float32).transpose(0, 2, 1)).astype(np16)
    WoT = np.ascontiguousarray(
        np.asarray(inputs["W_out"], np.float32).T).astype(np16)
    binr = np.tile(np.asarray(inputs["b_in"], np.float32)[None, :], (P, 1))
    boutr = np.tile(np.asarray(inputs["b_out"], np.float32)[None, :], (P, 1))
    maps = []
    for k in range(W):
        xk = np.zeros((cfg.Fp, cfg.NLp), np16)
        xk[:cfg.F, :NL] = x[k * NL:(k + 1) * NL].T.astype(np16)
        binl = binr.copy()
        binl[NL % P if NL % P else 0:] = 0.0   # pad-node rows of last tile
        m = dict(
            xT=xk, WinT=WinT, W1T=W1T, W2T=W2T, WfT=WfT, WoutT=WoT,
            b_in_rep=np.ascontiguousarray(binr, np.float32),
            b_in_lst=np.ascontiguousarray(binl, np.float32),
            b_out_rep=np.ascontiguousarray(boutr, np.float32),
            ixa=per_core[k]["ixa"], ixb=per_core[k]["ixb"],
            oh1=per_core[k]["oh1"], oh2=per_core[k]["oh2"],
            gcnt=per_core[k]["gcnt"])
        maps.append(m)
    return maps


def _install_profile_hook():
    """Provide antenv.axon_hooks (absent in this image) so that
    run_bass_kernel_spmd(trace=True) can collect an NTFF profile."""
    try:
        import types
        import antenv
        if "antenv.axon_hooks" not in sys.modules:
            mod = types.ModuleType("antenv.axon_hooks")
            state = {"hook": None}
            mod.set_axon_ntff_profile_hook = lambda h: state.__setitem__("hook", h)
            mod.get_axon_ntff_profile_hook = lambda: state["hook"]
            sys.modules["antenv.axon_hooks"] = mod
            antenv.axon_hooks = mod
        from antenv.axon_hooks import (get_axon_ntff_profile_hook,
                                       set_axon_ntff_profile_hook)
        if get_axon_ntff_profile_hook() is None:
            from trn_agent_boot.trn_boot import _ntff_profile_via_ctypes
            set_axon_ntff_profile_hook(
                _ntff_profile_via_ctypes("/opt/axon/libaxon_pjrt.so"))
        return True
    except Exception as e:  # degrade to untraced run
        print(f"profile hook unavailable: {e}")
        return False


def kernel(**inputs):
    cfg = Cfg()
    edge_index = np.asarray(inputs["edge_index"])
    meta, per_core = plan(cfg, edge_index)
    nc = build(cfg, meta)
    in_maps = host_inputs(cfg, meta, per_core, inputs)
    trace = bool(int(os.environ.get("GNN_TRACE", "0")))
    if trace:
        trace = _install_profile_hook()
    from concourse import bass_utils
    res = bass_utils.run_bass_kernel_spmd(
        nc, in_maps, core_ids=list(range(cfg.W)), trace=trace)
    if res.exec_time_ns is not None:
        print(f"HW exec time: {res.exec_time_ns} ns")
    outs = [res.results[k]["out"][:cfg.NL] for k in range(cfg.W)]
    return np.concatenate(outs, axis=0).astype(np.float32)

